# revision 4
# baseline (speedup 1.0000x reference)
"""Multi-head attention (B=4, N=2048, DIM=768, H=8, DH=96) on 8 TRN2 NeuronCores.

Sharding: (batch, head)-parallel. Core c handles batch c//2 and the 4 heads
Hs = [0..3] (even c) or [4..7] (odd c) — selected host-side by weight slicing,
so the kernel is SPMD-identical. Each core computes K/V/Q for its 4 heads over
ALL 2048 tokens (no K/V duplication — that's the win vs data-parallel), runs
attention as 8 "virtual heads" vh = (head wh=vh%4, query-half: vh<4 = the
PARTNER's output half, vh>=4 = OWN half), then projects its OWN query half
using all 8 heads' normalized O: 4 local + 4 received from the pair core.

O-exchange: after each vh<4 completes, its normalized O^T [97,1024] bf16 is
ReduceScattered pairwise (replica_groups [[0,1],..]) with the payload
duplicated in both input shards; received = RS_out - own_payload (one DVE
subtract) recovers the partner's payload SPMD-symmetrically (~0.3% bf16
rounding noise). Collectives run on TOPSP/SDMA silicon and overlap freely
with compute; the 4th RS lands ~2 windows before the projection needs it.

Per-core compute (all matmuls bf16, fp32 PSUM accumulation):
  - Dense Q^T/K^T projection per head (f-dim zero-padded 96->128), direct
    PSUM->SBUF eviction. K^T [128,2048] per head persists in SBUF (reused by
    both query halves).
  - V projection in natural space; slot 0 of each per-head V tile is a
    constant 1.0 column (1|V) so the attn@V matmul also produces the softmax
    row-sums in PSUM row 0.
  - dots P^T[nk,nq]: lhsT=K^T[96dh, 128nk], rhs=Q^T[96dh, 512nq]; softmax
    scale folded into w_q host-side. Two 512-query chunks land in one
    [128,1024] PSUM pair; ONE exp() on ScalarE PSUM->SBUF(bf16) covers both.
    No max subtraction (logits max ~9, fp32-safe).
  - O'^T[97, nq] accumulated over 16 key tiles; row 0 = row-sum s.
  - normalize: evacuate O' to SBUF, reciprocal of row 0, gpsimd
    partition_broadcast, single multiply (row 0 becomes 1.0 = the bias row).
  - proj y^T[c,nq] over 8 O-source slots (0-3 local own-half heads, 4-7
    received partner heads; host permutes wp to match): slots
    {0,1,2,4,5,6,7} accumulated as fillers during vh7's attention (bias
    folded into wp slot-0 row 0), slot 3 (vh7's O) added at the tail.

Input DMAs are split across the sync and scalar HW DGE queues. Output per
core: y^T [768, 1024] bf16 (its own query half); host reassembles.
"""

import numpy as np
import ml_dtypes

B, N, DIM = 4, 2048, 768
H, DH = 8, 96
HL = 4             # local heads per core
NQ = N // 2        # query rows per core output
SCALE = DH ** -0.5
NCORES = 8
CT = DIM // 128    # 6 contraction chunks
NT = N // 128      # 16 key tiles
NQC = NQ // 512    # 2 query chunks of 512 per half
NVH = 8            # virtual heads = 4 local heads x 2 query halves

_CACHE = {}


def _build():
    import concourse.mybir as mybir
    import concourse.tile as tile
    from concourse import bacc

    f32 = mybir.dt.float32
    bf16 = mybir.dt.bfloat16
    Exp = mybir.ActivationFunctionType.Exp
    mult = mybir.AluOpType.mult
    sub = mybir.AluOpType.subtract

    nc = bacc.Bacc("TRN2", debug=False, num_devices=NCORES)

    xta_d = nc.dram_tensor("xta", [128, CT, NQ], bf16, kind="ExternalInput")
    xtb_d = nc.dram_tensor("xtb", [128, CT, NQ], bf16, kind="ExternalInput")
    wq_d = nc.dram_tensor("wq", [128, CT, HL * 128], bf16, kind="ExternalInput")
    wk_d = nc.dram_tensor("wk", [128, CT, HL * 128], bf16, kind="ExternalInput")
    wv_d = nc.dram_tensor("wv", [128, CT, HL * DH], bf16, kind="ExternalInput")
    wp_d = nc.dram_tensor("wp", [DH + 1, H, DIM], bf16, kind="ExternalInput")
    out_d = nc.dram_tensor("out", [DIM, NQ], bf16, kind="ExternalOutput")

    RG = [[0, 1], [2, 3], [4, 5], [6, 7]]

    with tile.TileContext(nc) as tc:
        with (
            tc.tile_pool(name="const", bufs=1) as cpool,
            tc.tile_pool(name="ptp", bufs=3) as pt_pool,
            tc.tile_pool(name="onp", bufs=14) as on_pool,
            tc.tile_pool(name="smallp", bufs=2) as small_pool,
            tc.tile_pool(name="stagep", bufs=3) as stage_pool,
            tc.tile_pool(name="ysb", bufs=4) as y_pool,
            tc.tile_pool(name="ccd", bufs=1, space="DRAM") as dram_pool,
            tc.tile_pool(name="ps_qkv", bufs=2, space="PSUM") as psum_qkv,
            tc.tile_pool(name="ps_d", bufs=2, space="PSUM") as psum_d,
            tc.tile_pool(name="ps_o", bufs=2, space="PSUM") as psum_o,
        ):
            # ---- persistent SBUF tensors, consolidated input DMAs ----
            xt_a = cpool.tile([128, CT, NQ], bf16, name="xt_a")
            xt_b = cpool.tile([128, CT, NQ], bf16, name="xt_b")
            wk_sb = cpool.tile([128, CT, HL * 128], bf16, name="wk_sb")
            wq_sb = cpool.tile([128, CT, HL * 128], bf16, name="wq_sb")
            wv_sb = cpool.tile([128, CT, HL * DH], bf16, name="wv_sb")
            wp_sb = cpool.tile([DH + 1, H, DIM], bf16, name="wp_sb")
            # K^T per local head, persistent (used by vh and vh+4)
            kt_sb = [cpool.tile([128, N], bf16, name=f"kt{w}") for w in range(HL)]
            qt_sb = {}   # vh -> [128, NQ] tile (rotating)
            v_sb = [cpool.tile([128, HL, DH + 1], bf16, name=f"v{t}") for t in range(NT)]
            y1_sb = [
                [cpool.tile([128, 512], bf16, name=f"y1_{ct}_{qc}") for qc in range(NQC)]
                for ct in range(CT)
            ]

            # inputs are pre-arranged partition-major on the host: every DMA
            # is a fully-contiguous per-partition transfer. Split across the
            # sync and scalar HW DGE queues, ordered by first use on each.
            nc.sync.dma_start(wk_sb[:], wk_d.ap())
            nc.scalar.dma_start(xt_a[:], xta_d.ap())
            nc.sync.dma_start(wq_sb[:], wq_d.ap())
            nc.scalar.dma_start(wv_sb[:], wv_d.ap())
            nc.sync.dma_start(xt_b[:], xtb_d.ap())
            nc.scalar.dma_start(wp_sb[:], wp_d.ap())

            for t in range(NT):
                nc.vector.memset(v_sb[t][:, :, 0:1], 1.0)

            # identity for re-loading y1 into PSUM at the tail
            id_d = nc.inline_tensor(
                np.eye(128, dtype=ml_dtypes.bfloat16), name="id128"
            )
            id_sb = cpool.tile([128, 128], bf16, name="id_sb")
            nc.sync.dma_start(id_sb[:], id_d.ap())

            # PE warmup: keep the TensorEngine busy through the input-DMA
            # window so the HAM clock gate is at 8/8 when real work starts.
            warm_sb = cpool.tile([128, 128], bf16, name="warm_sb")
            nc.vector.memset(warm_sb[:], 0.0)
            ones_sb = cpool.tile([1, 128], f32, name="ones_sb")
            nc.vector.memset(ones_sb[:], 1.0)
            warm_ps = psum_qkv.tile([128, 128], f32, name="warmps", tag="qkvps")
            for _ in range(60):
                nc.tensor.matmul(
                    warm_ps, lhsT=warm_sb[:], rhs=warm_sb[:], start=True, stop=True
                )

            def xt_cols(lo):
                src = xt_a if lo < NQ else xt_b
                off = lo if lo < NQ else lo - NQ
                return src, off

            # ---- head-padded Q/K projection chunks (direct eviction) ----
            def k_chunk(wh, nc_):
                src, off = xt_cols(nc_ * 512)
                ps = psum_qkv.tile([128, 512], f32, name="kps", tag="qkvps")
                for ct in range(CT):
                    nc.tensor.matmul(
                        ps,
                        lhsT=wk_sb[:, ct, wh * 128:(wh + 1) * 128],
                        rhs=src[:, ct, off:off + 512],
                        start=(ct == 0),
                        stop=(ct == CT - 1),
                    )
                nc.vector.tensor_copy(
                    out=kt_sb[wh][:, nc_ * 512:(nc_ + 1) * 512], in_=ps[:]
                )

            def q_chunk(vh, qc):
                if qc == 0:
                    qt_sb[vh] = stage_pool.tile(
                        [128, NQ], bf16, name="qt", tag="qt", bufs=3
                    )
                wh = vh % HL
                half = 0 if vh < HL else 1
                src, off = xt_cols(half * NQ + qc * 512)
                ps = psum_qkv.tile([128, 512], f32, name="qps", tag="qkvps")
                for ct in range(CT):
                    nc.tensor.matmul(
                        ps,
                        lhsT=wq_sb[:, ct, wh * 128:(wh + 1) * 128],
                        rhs=src[:, ct, off:off + 512],
                        start=(ct == 0),
                        stop=(ct == CT - 1),
                    )
                nc.vector.tensor_copy(
                    out=qt_sb[vh][:, qc * 512:(qc + 1) * 512], in_=ps[:]
                )

            def v_chunk(t):
                src, off = xt_cols(t * 128)
                ps = psum_qkv.tile([128, 512], f32, name="vps", tag="qkvps")
                vps = ps[:, :HL * DH]
                for ct in range(CT):
                    nc.tensor.matmul(
                        vps,
                        lhsT=src[:, ct, off:off + 128],
                        rhs=wv_sb[:, ct, :],
                        start=(ct == 0),
                        stop=(ct == CT - 1),
                    )
                # single strided eviction into the 4 per-head [*,1:97] slots
                nc.vector.tensor_copy(
                    out=v_sb[t][:, :, 1:DH + 1],
                    in_=vps.rearrange("p (a b) -> p a b", a=HL),
                )

            # O-source slots for the projection: slot j<4 = local own-half
            # head j (vh 4+j), slot 4+w = received partner head (recv[w]).
            on_sb = {}

            def proj_part(ct, qc):
                """Accumulate slots {0,1,2,4,5,6,7}; bias folded into slot 0."""
                yp = psum_qkv.tile([128, 512], f32, name="yps", tag="qkvps")
                srcs = [0, 1, 2, 4, 5, 6, 7]
                for i, j in enumerate(srcs):
                    nc.tensor.matmul(
                        yp,
                        lhsT=wp_sb[:, j, ct * 128:(ct + 1) * 128],
                        rhs=on_sb[(j, qc)][:],
                        start=(i == 0),
                        stop=(i == len(srcs) - 1),
                    )
                nc.vector.tensor_copy(out=y1_sb[ct][qc][:], in_=yp[:])

            def proj_tail(ct, use_scalar):
                # spread the six ct chunks over all three (now dead) PSUM
                # pools so the chains overlap instead of serializing on one
                # 2-buffer ring.
                y_sb = y_pool.tile([128, 1024], bf16, name="y", tag="y")
                if ct % 3 == 0:
                    yp = psum_d.tile([128, 1024], f32, name="yp7", tag="dps")
                    for qc in range(NQC):
                        yps = yp[:, qc * 512:(qc + 1) * 512]
                        nc.tensor.matmul(
                            yps, lhsT=id_sb[:], rhs=y1_sb[ct][qc][:],
                            start=True, stop=False,
                        )
                        nc.tensor.matmul(
                            yps,
                            lhsT=wp_sb[:, 3, ct * 128:(ct + 1) * 128],
                            rhs=on_sb[(3, qc)][:],
                            start=False,
                            stop=True,
                        )
                    if use_scalar:
                        nc.scalar.copy(y_sb[:], yp[:])
                    else:
                        nc.vector.tensor_copy(out=y_sb[:], in_=yp[:])
                else:
                    pool, tag = (
                        (psum_qkv, "qkvps") if ct % 3 == 1 else (psum_o, "ops")
                    )
                    for qc in range(NQC):
                        yps = pool.tile([128, 512], f32, name="yp7n", tag=tag)
                        nc.tensor.matmul(
                            yps, lhsT=id_sb[:], rhs=y1_sb[ct][qc][:],
                            start=True, stop=False,
                        )
                        nc.tensor.matmul(
                            yps,
                            lhsT=wp_sb[:, 3, ct * 128:(ct + 1) * 128],
                            rhs=on_sb[(3, qc)][:],
                            start=False,
                            stop=True,
                        )
                        half = y_sb[:, qc * 512:(qc + 1) * 512]
                        if qc == 0:
                            nc.scalar.copy(half, yps[:])
                        else:
                            nc.vector.tensor_copy(out=half, in_=yps[:])
                dma_eng = nc.sync if ct % 2 == 0 else nc.scalar
                dma_eng.dma_start(
                    out_d.ap()[ct * 128:(ct + 1) * 128, :], y_sb[:]
                )

            # ---- pairwise O-exchange: RS with duplicated payload ----
            def exchange(wh, pay):
                in_b = dram_pool.tile(
                    [2 * (DH + 1), NQ], bf16, name=f"ccin{wh}", tag=f"ccin{wh}"
                )
                out_b = dram_pool.tile(
                    [DH + 1, NQ], bf16, name=f"ccout{wh}", tag=f"ccout{wh}"
                )
                nc.gpsimd.dma_start(in_b[0:DH + 1, :], pay[:])
                nc.gpsimd.dma_start(in_b[DH + 1:, :], pay[:])
                nc.gpsimd.collective_compute(
                    "ReduceScatter",
                    mybir.AluOpType.add,
                    replica_groups=RG,
                    ins=[in_b.opt()],
                    outs=[out_b.opt()],
                )
                rsum = on_pool.tile([DH + 1, NQ], bf16, name="rsum", tag="rsum", bufs=2)
                nc.gpsimd.dma_start(rsum[:], out_b[:])
                recv = on_pool.tile([DH + 1, NQ], bf16, name="recv", tag=f"recv{wh}", bufs=1)
                nc.vector.tensor_tensor(recv[:], rsum[:], pay[:], sub)
                for qc in range(NQC):
                    on_sb[(4 + wh, qc)] = recv[:, qc * 512:(qc + 1) * 512]

            # ---- attention for one virtual head, fillers interleaved ----
            def attn_head(vh, fillers):
                wh = vh % HL
                o_ps = [
                    psum_o.tile([DH + 1, 512], f32, name=f"ops{qc}", tag="ops")
                    for qc in range(NQC)
                ]
                for t in range(NT):
                    d_ps = psum_d.tile([128, 1024], f32, name="dps", tag="dps")
                    for qc in range(NQC):
                        nc.tensor.matmul(
                            d_ps[:, qc * 512:(qc + 1) * 512],
                            lhsT=kt_sb[wh][:, t * 128:(t + 1) * 128],
                            rhs=qt_sb[vh][:, qc * 512:(qc + 1) * 512],
                            start=True,
                            stop=True,
                        )
                    pt = pt_pool.tile([128, 1024], bf16, name="pt", tag="pt")
                    nc.scalar.activation(pt[:], d_ps[:], Exp)
                    for qc in range(NQC):
                        nc.tensor.matmul(
                            o_ps[qc],
                            lhsT=v_sb[t][:, wh, :],
                            rhs=pt[:, qc * 512:(qc + 1) * 512],
                            start=(t == 0),
                            stop=(t == NT - 1),
                        )
                    for fn in fillers.get(t, ()):
                        fn()
                # PSUM evacuation copies run immediately (frees the o_ps
                # banks); the rest of the normalize chain is returned as a
                # closure and emitted mid-way through a LATER head, far from
                # the congested head boundary.
                o_sts = []
                for qc in range(NQC):
                    o_st = small_pool.tile(
                        [DH + 1, 512], f32, name="ostage", tag="ostage", bufs=7
                    )
                    nc.vector.tensor_copy(out=o_st[:], in_=o_ps[qc][:])
                    o_sts.append(o_st)

                def finish_normalize(vh=vh, wh=wh, o_sts=o_sts):
                    if vh == NVH - 1:
                        for qc in range(NQC):
                            # PE broadcast (fp32 matmul): sub-us vs 1us on
                            # gpsimd, and the PE is otherwise idle here
                            rsb = small_pool.tile(
                                [1, 512], f32, name="rsb", tag="rs", bufs=4
                            )
                            nc.vector.reciprocal_approx_fast(
                                out=rsb[:], in_=o_sts[qc][0:1, :]
                            )
                            bps = psum_o.tile(
                                [DH + 1, 512], f32, name="bps", tag="ops"
                            )
                            nc.tensor.matmul(
                                bps,
                                lhsT=ones_sb[0:1, 0:DH + 1],
                                rhs=rsb[:],
                                start=True,
                                stop=True,
                            )
                            on = on_pool.tile(
                                [DH + 1, 512], bf16, name="on", tag="on", bufs=2
                            )
                            on_sb[(3, qc)] = on
                            nc.vector.tensor_tensor(on[:], o_sts[qc][:], bps[:], mult)
                        return
                    # gpsimd partition_broadcast flavor; one [97,1024] tile
                    pay = on_pool.tile(
                        [DH + 1, NQ], bf16, name="onw", tag="onw", bufs=7
                    )
                    for qc in range(NQC):
                        rs = small_pool.tile([1, 512], f32, name="rs", tag="rs", bufs=4)
                        nc.vector.reciprocal_approx_fast(
                            out=rs[:], in_=o_sts[qc][0:1, :]
                        )
                        sb = small_pool.tile(
                            [DH + 1, 512], f32, name="sbc", tag="sbc", bufs=4
                        )
                        nc.gpsimd.partition_broadcast(sb[:], rs[:])
                        # row 0 becomes s*(1/s) = 1.0 -> the bias row
                        nc.vector.tensor_tensor(
                            pay[:, qc * 512:(qc + 1) * 512], o_sts[qc][:], sb[:], mult
                        )
                    if vh < HL:
                        # partner-destined: ship it
                        exchange(wh, pay)
                    else:
                        for qc in range(NQC):
                            on_sb[(vh - HL, qc)] = pay[:, qc * 512:(qc + 1) * 512]

                return finish_normalize

            # ---- software-pipelined emission ----
            k_chunk(0, 0)
            q_chunk(0, 0)
            q_chunk(0, 1)
            v_chunk(0)
            v_chunk(1)

            def mk_fillers(vh):
                f = {}

                def addf(slot, fn):
                    f.setdefault(slot, []).append(fn)

                if vh == 0:
                    for nc_ in (1, 2, 3):
                        addf(4 * nc_ - 3, (lambda n=nc_: k_chunk(0, n)))
                    for t in range(2, NT):
                        addf(t - 2, lambda tt=t: v_chunk(tt))
                # remaining heads' K during vh0-2 (kt persists; vh+4 reuses)
                if vh < HL - 1:
                    for i, nc_ in enumerate((0, 1, 2, 3)):
                        addf(2 * i + 2, (lambda hh=vh + 1, n=nc_: k_chunk(hh, n)))
                # next vh's Q
                if vh + 1 < NVH:
                    addf(11, lambda hh=vh + 1: q_chunk(hh, 0))
                    addf(13, lambda hh=vh + 1: q_chunk(hh, 1))
                # slots {0,1,2,4,5,6,7} projection during the last vh
                if vh == NVH - 1:
                    for i in range(12):
                        ct, qc = i // 2, i % 2
                        addf(i + 4, lambda c=ct, q=qc: proj_part(c, q))
                return f

            from collections import deque
            pending_norm = deque()
            for vh in range(NVH):
                f = mk_fillers(vh)
                if vh == NVH - 1:
                    # last head: all previous normalizes must land before
                    # the projection fillers (slots 4+)
                    while pending_norm:
                        f.setdefault(2, []).append(pending_norm.popleft())
                elif len(pending_norm) >= 2:
                    f.setdefault(3, []).insert(0, pending_norm.popleft())
                pending_norm.append(attn_head(vh, f))

            # ---- tail: vh7 normalize + slot-3 projection + combine + out ----
            # keep the PE warm through the normalize window so the proj_tail
            # matmuls run at full clock: dummy matmuls gated on vh7's
            # evacuated O' (so the scheduler can't hoist them earlier).
            h7_norm = pending_norm.popleft()
            for _ in range(10):
                wps = psum_o.tile([DH + 1, 512], f32, name="wps", tag="ops")
                nc.tensor.matmul(
                    wps,
                    lhsT=warm_sb[0:DH + 1, 0:DH + 1],
                    rhs=qt_sb[NVH - 1][0:DH + 1, 0:512],
                    start=True,
                    stop=True,
                )
            h7_norm()
            for ct in range(CT):
                proj_tail(ct, use_scalar=(ct % 2 == 1))

    nc.compile()
    return nc


def _get_nc():
    if "nc" not in _CACHE:
        _CACHE["nc"] = _build()
    return _CACHE["nc"]


def _prep_shards(x, w_qkv, w_proj, b_proj):
    bf16 = ml_dtypes.bfloat16
    x = np.asarray(x, dtype=np.float32)
    w_qkv = np.asarray(w_qkv, dtype=np.float32)
    w_proj = np.asarray(w_proj, dtype=np.float32)
    b_proj = np.asarray(b_proj, dtype=np.float32)

    def pmajor(w):  # [768(c), F] -> [128, CT, F] partition-major contiguous
        return np.ascontiguousarray(
            w.reshape(CT, 128, w.shape[1]).transpose(1, 0, 2)
        ).astype(bf16)

    def pad_heads4(w, heads):  # [768(c), 768(f)] -> [768, 4*128] zero-padded
        wp_ = np.zeros((DIM, HL, 128), np.float32)
        wp_[:, :, :DH] = w.reshape(DIM, H, DH)[:, heads, :]
        return wp_.reshape(DIM, HL * 128)

    wq_t = w_qkv[0:DIM].T * SCALE           # [768(c), 768(f)]
    wk_t = w_qkv[DIM:2 * DIM].T
    wv_t = w_qkv[2 * DIM:3 * DIM].T
    wp_heads = w_proj.T.reshape(H, DH, DIM)  # [H, DH, DIM]

    in_maps = []
    for c in range(NCORES):
        b, parity = divmod(c, 2)
        Hs = list(range(0, HL)) if parity == 0 else list(range(HL, H))
        Ho = list(range(HL, H)) if parity == 0 else list(range(0, HL))

        xt = x[b].T  # [768, 2048]
        # arrange columns: [0:1024] = PARTNER's output half, [1024:2048] = OWN
        if parity == 0:
            xt = np.concatenate([xt[:, NQ:], xt[:, :NQ]], axis=1)
        wq_b = pmajor(pad_heads4(wq_t, Hs))
        wk_b = pmajor(pad_heads4(wk_t, Hs))
        wv_b = pmajor(
            np.ascontiguousarray(
                wv_t.reshape(DIM, H, DH)[:, Hs, :]
            ).reshape(DIM, HL * DH)
        )
        # wp slots: j<4 = own head Hs[j], j>=4 = partner head Ho[j-4];
        # row 0 = bias (slot 0 only), rows 1..DH = weights
        wp_arr = np.zeros((DH + 1, H, DIM), np.float32)
        for j, hh in enumerate(Hs + Ho):
            wp_arr[1:DH + 1, j, :] = wp_heads[hh]
        wp_arr[0, 0, :] = b_proj
        in_maps.append({
            "xta": pmajor(xt[:, 0:NQ]),
            "xtb": pmajor(xt[:, NQ:N]),
            "wq": wq_b,
            "wk": wk_b,
            "wv": wv_b,
            "wp": np.ascontiguousarray(wp_arr).astype(bf16),
        })
    return in_maps


def kernel(x, w_qkv, w_proj, b_proj):
    from concourse.bass_utils import run_bass_kernel_spmd

    nc = _get_nc()
    in_maps = _prep_shards(x, w_qkv, w_proj, b_proj)
    res = run_bass_kernel_spmd(nc, in_maps, core_ids=list(range(NCORES)))
    out = np.empty((B, N, DIM), np.float32)
    for c in range(NCORES):
        b, half = divmod(c, 2)
        yT = np.asarray(res.results[c]["out"], dtype=np.float32)  # [768, 1024]
        out[b, half * NQ:(half + 1) * NQ, :] = yT.T
    return out
